# revision 16
# baseline (speedup 1.0000x reference)
"""Multi-head attention (B=2, S=2048, d_model=768, 12 heads) on 8 trn2 cores.

Sharding: 24 (batch, head) pairs -> 3 heads + 1 batch per core; host sums
the 4 per-core partial output projections of each batch and adds b_o.

v2 redesign over the fp32r baseline (which measured dma 81us + compute
150us ~= full 237us, i.e. zero DMA/compute overlap on HW):

  - bf16 data path everywhere (x, weights, Q^T/K^T, V, est, O^T, out);
    PSUM accumulation stays fp32.  Halves HBM traffic (20.5 -> 9.9 MB per
    body); matmul cost per column is unchanged but fp32r small-tile
    penalties disappear.
  - One continuous software-pipelined stream across ALL reps: tile pools
    are global, and each (body, qc) chunk's score/exp groups are
    interleaved with "filler" PE tasks — the previous chunk's output
    projection + store, the next chunk's Q projection, and (in the last
    chunk of a body) the NEXT body's V0/K0/Q0 front — so ScalarE (exp,
    the 66us/body floor) and PE (80us/body) both stay near-saturated
    through body boundaries.
  - All HBM transfers use the SP HWDGE ring; stores are emitted as
    stream fillers mid-chunk, so in steady state the next body's loads
    are already in flight ahead of them.  (The Pool/gpsimd SWDGE ring
    measured ~150us/body SLOWER for the 4 stores — Q7 descriptor
    generation is far costlier on HW than the cost model's 1.2us; the
    Activation HWDGE ring head-of-line blocks exp dispatch.  GPSIMD also
    cannot read PSUM, so all PSUM->SBUF copies live on DVE, except the
    reciprocal-broadcast staging copy which rides ScalarE as a Copy
    activation.)
  - V is projected directly in [keys, dk] orientation (lhsT = x^T chunk,
    rhs = W_v), eliminating the PE transpose pass; the mask column that
    rides the P@V matmul as the softmax denominator is appended on DVE.
  - The 16 per-qt output stores are batched into 4 per-qc stores of
    [512, 768] via a [128, 4, 768] staging tile.
  - Key compaction as before: masked keys dropped on host, kbl = ceil/128.

The TPB instruction encoding holds a single sync-wait slot; _legalize_sync
splits extra waits into single-wait NoOps on the same in-order queue.
"""

import json
import sys

for _p in ("/opt/trn_rl_repo",):
    if _p not in sys.path:
        sys.path.insert(0, _p)

import numpy as np

import concourse.bass as bass
import concourse.mybir as mybir
from concourse.tile import TileContext
from concourse.bass_utils import run_bass_kernel_spmd

D_MODEL = 768
N_HEADS = 12
DK = 64
B = 2
SQ = 2048
SK = 2048
HPC = 3  # heads per core
N_CORES = 8
FC = D_MODEL // 128  # 6 f-chunks of 128
QC = SQ // 512  # 4 query chunks of 512

F32 = mybir.dt.float32
F32R = mybir.dt.float32r
BF16 = mybir.dt.bfloat16


def _legalize_sync(bj):
    """Split >1-wait instructions into single-wait NoOps + the instruction."""
    n = 0
    for fn in bj["functions"]:
        for blk in fn["blocks"]:
            out = []
            for inst in blk["instructions"]:
                si = inst.get("sync_info") or None
                waits = (si or {}).get("on_wait") or []
                if len(waits) > 1:
                    merged = {}
                    for w in waits:
                        k = w.get("id", w.get("ant_name"))
                        if k not in merged or w.get("wait_value", 0) > merged[
                            k
                        ].get("wait_value", 0):
                            merged[k] = w
                    waits = list(merged.values())
                if len(waits) > 1:
                    for w in waits[:-1]:
                        n += 1
                        out.append(
                            {
                                "engine": inst["engine"],
                                "ins": [],
                                "name": f"I-syncfix-{n}",
                                "opcode": "NoOp",
                                "outs": [],
                                "sync_info": {"on_update": [], "on_wait": [w]},
                            }
                        )
                    si["on_wait"] = [waits[-1]]
                out.append(inst)
            blk["instructions"] = out
    return bj


class _Bass(bass.Bass):
    def to_json_bytes(self):
        bj = json.loads(super().to_json_bytes())
        return json.dumps(_legalize_sync(bj)).encode()


def _emit_weights(nc, T, kbl, singles):
    wv_sb = singles.tile([128, FC, HPC * DK], BF16)
    bv_sb = singles.tile([1, HPC * DK], BF16)
    m01_sb = singles.tile([128, kbl], F32)
    wk_sb = singles.tile([128, FC, 2, 128], BF16)
    bk_sb = singles.tile([128, 2], F32)
    wq_sb = singles.tile([128, FC, 2, 128], BF16)
    bq_sb = singles.tile([128, 2], F32)
    onesr_sb = singles.tile([1, 128], F32R)
    onesb_sb = singles.tile([1, 128], BF16)
    wo_sb = singles.tile([128, 2 * D_MODEL], BF16)
    nc.sync.dma_start(
        out=wv_sb, in_=T["wv"].rearrange("p (a m) -> p a m", a=FC))
    nc.sync.dma_start(out=bv_sb, in_=T["bv"][:])
    nc.sync.dma_start(out=m01_sb, in_=T["m01"].rearrange("(t p) -> p t", p=128))
    nc.sync.dma_start(out=onesr_sb, in_=T["onesr"][:])
    nc.sync.dma_start(out=onesb_sb, in_=T["onesb"][:])
    nc.sync.dma_start(
        out=wk_sb, in_=T["wk"].rearrange("p (a b c) -> p a b c", a=FC, b=2))
    nc.sync.dma_start(out=bk_sb, in_=T["bk"][:])
    nc.sync.dma_start(
        out=wq_sb, in_=T["wq"].rearrange("p (a b c) -> p a b c", a=FC, b=2))
    nc.sync.dma_start(out=bq_sb, in_=T["bq"][:])
    nc.sync.dma_start(out=wo_sb, in_=T["wo"][:])
    return dict(wv_sb=wv_sb, bv_sb=bv_sb, m01_sb=m01_sb,
                wk_sb=wk_sb, bk_sb=bk_sb, wq_sb=wq_sb, bq_sb=bq_sb,
                onesr_sb=onesr_sb, onesb_sb=onesb_sb, wo_sb=wo_sb)


def _emit_all(nc, T, kbl, W, P, reps, mode="full", store_ring="sp"):
    """Emit `reps` kernel executions as one continuous pipelined stream."""
    do_dma = mode in ("full", "dma")
    do_compute = mode in ("full", "compute")
    skc = kbl * 128
    kslices = [(0, 128)]
    off = 128
    while off < skc:
        w = min(512, skc - off)
        kslices.append((off, w))
        off += w

    wv_sb = W["wv_sb"]; bv_sb = W["bv_sb"]; m01_sb = W["m01_sb"]
    wk_sb = W["wk_sb"]; bk_sb = W["bk_sb"]; wq_sb = W["wq_sb"]
    bq_sb = W["bq_sb"]; onesr_sb = W["onesr_sb"]; onesb_sb = W["onesb_sb"]
    wo_sb = W["wo_sb"]
    pers = P["pers"]; xqp = P["xq"]; xkvp = P["xkv"]; exps = P["exps"]
    rcps = P["rcp"]; outs = P["outs"]
    px = P["px"]; pst = P["pst"]; po = P["po"]

    def load_q_slice(qc):
        t = xqp.tile([128, FC, 512], BF16, tag="xq", name="xq")
        if do_dma:
            nc.sync.dma_start(
                out=t,
                in_=T["xtq"].rearrange("(a p) q -> p a q", p=128)[
                    :, :, qc * 512 : (qc + 1) * 512
                ],
            )
        else:
            nc.gpsimd.memset(t[:, 0, 0:2], 0.0)
        return t

    def load_kv_slice(name, off, w):
        t = xkvp.tile([128, FC, w], BF16, tag="x" + name, name="xkv")
        if do_dma:
            nc.sync.dma_start(
                out=t,
                in_=T[name].rearrange("(a p) k -> p a k", p=128)[
                    :, :, off : off + w
                ],
            )
        else:
            nc.gpsimd.memset(t[:, 0, 0:2], 0.0)
        return t

    def new_body():
        """Allocate a body's activation tiles and issue its loads."""
        bd = {}
        bd["qt"] = pers.tile([128, 2, SQ], BF16, tag="qt", name="qt")
        bd["kt"] = pers.tile([128, 2, skc], BF16, tag="kt", name="kt")
        bd["vaug"] = pers.tile(
            [128, kbl, HPC, DK + 1], BF16, tag="vaug", name="vaug")
        bd["ot"] = pers.tile([128, 2, SQ], BF16, tag="ot", name="ot")
        bd["xvs"] = [load_kv_slice("xtv", off, w) for off, w in kslices[:1]]
        bd["xks"] = [load_kv_slice("xtk", off, w) for off, w in kslices[:1]]
        bd["xqs"] = [load_q_slice(0)]
        for off, w in kslices[1:]:
            bd["xvs"].append(load_kv_slice("xtv", off, w))
            bd["xks"].append(load_kv_slice("xtk", off, w))
        for qc in range(1, QC):
            bd["xqs"].append(load_q_slice(qc))
        return bd

    def v_block(bd, i, kt):
        """V block kt (from x slice i) -> masked vaug + mask column,
        directly in [keys, dk] orientation; bias via a K=1 ones matmul."""
        loc = (kt - kslices[i][0] // 128) * 128
        x_t = bd["xvs"][i]
        ps = px.tile([128, HPC, DK], F32, tag="ps", name="ps")
        for fc in range(FC):
            nc.tensor.matmul(
                ps,
                x_t[:, fc, loc : loc + 128],
                wv_sb[:, fc, :],
                start=(fc == 0),
                stop=False,
            )
        nc.tensor.matmul(
            ps, onesb_sb[0:1, 0:128], bv_sb[0:1, :], start=False, stop=True
        )
        nc.vector.tensor_scalar_mul(
            bd["vaug"][:, kt, :, 0:DK], ps, m01_sb[:, kt : kt + 1]
        )
        mcol = m01_sb[:, kt : kt + 1]
        bcast = bass.AP(
            tensor=mcol.tensor,
            offset=mcol.offset,
            ap=[mcol.ap[0], [0, HPC], [0, 1]],
        )
        nc.gpsimd.tensor_copy(bd["vaug"][:, kt, :, DK : DK + 1], bcast)

    def proj_k_ch(bd, i, ch):
        off, w = kslices[i]
        ps = px.tile([128, 512], F32, tag="ps", name="ps")
        for fc in range(FC):
            nc.tensor.matmul(
                ps[:, 0:w],
                wk_sb[:, fc, ch, :],
                bd["xks"][i][:, fc, :],
                start=(fc == 0),
                stop=(fc == FC - 1),
            )
        nc.vector.tensor_scalar_add(
            bd["kt"][:, ch, off : off + w], ps[:, 0:w], bk_sb[:, ch : ch + 1])

    def proj_q_ch(bd, qc, ch):
        ps = px.tile([128, 512], F32, tag="ps", name="ps")
        for fc in range(FC):
            nc.tensor.matmul(
                ps,
                wq_sb[:, fc, ch, :],
                bd["xqs"][qc][:, fc, :],
                start=(fc == 0),
                stop=(fc == FC - 1),
            )
        nc.vector.tensor_scalar_add(
            bd["qt"][:, ch, qc * 512 : (qc + 1) * 512], ps,
            bq_sb[:, ch : ch + 1])

    # ---- score unit: scores matmul + exp (emit_score), deferred PV
    def make_unit(bd, qsl, o_ps, vh, h_ch, h_half, kb, start, stop):
        state = {}

        def emit_score():
            ksl = slice(kb * 128, (kb + 1) * 128)
            r = slice(64, 128) if h_half else slice(0, 64)
            stp = pst.tile([128, 512], F32, tag="stp", name="stp")
            nc.tensor.matmul(
                stp,
                bd["kt"][r, h_ch, ksl],
                bd["qt"][r, h_ch, qsl],
                start=True,
                stop=True,
                tile_position=(64 if h_half else 0, 0),
            )
            est = exps.tile([128, 512], BF16, tag="est", name="est")
            nc.scalar.activation(
                est, stp, mybir.ActivationFunctionType.Exp, scale=0.125
            )
            state["est"] = est

        def emit_pv():
            nc.tensor.matmul(
                o_ps[0 : DK + 1, :],
                bd["vaug"][:, kb, vh, :],
                state["est"],
                start=start,
                stop=stop,
            )

        return (emit_score, emit_pv, kb)

    def build_units(bd, qsl, o0, o1, o2):
        units = []
        for kb in range(kbl):
            units.append(
                make_unit(bd, qsl, o0, 0, 0, 0, kb, kb == 0, kb == kbl - 1))
            units.append(
                make_unit(bd, qsl, o1, 1, 0, 1, kb, kb == 0, kb == kbl - 1))
            if kb % 2 == 1:
                kp = kb // 2
                units.append(
                    make_unit(bd, qsl, o2, 2, 1, 0, 2 * kp, kp == 0, False))
                units.append(
                    make_unit(
                        bd, qsl, o2, 2, 1, 1, 2 * kp + 1, False, kb == kbl - 1))
            if kb == kbl - 1 and kbl % 2 == 1:
                units.append(make_unit(bd, qsl, o2, 2, 1, 0, kb, kb == 0, True))
        return units

    def normalize(bd, h, qc, o_ps):
        """ot[...] = O'[0:64] * (1/rowsum); rowsum lives in row 64. The
        reciprocal is broadcast to 64 partitions via a K=1 ones matmul."""
        rs_rcp = rcps.tile([1, 512], F32R, tag="rs_rcp", name="rs_rcp")
        nc.vector.reciprocal(rs_rcp, o_ps[DK : DK + 1, :])
        rsm = px.tile([64, 512], F32, tag="ps", name="ps")
        nc.tensor.matmul(rsm, onesr_sb[0:1, 0:64], rs_rcp, start=True, stop=True)
        rcpm = rcps.tile([64, 512], F32, tag="rcpm", name="rcpm")
        nc.scalar.activation(rcpm, rsm, mybir.ActivationFunctionType.Copy)
        ch, r0 = ((0, 0), (0, 64), (1, 0))[h]
        nc.vector.tensor_mul(
            bd["ot"][r0 : r0 + 64, ch, qc * 512 : (qc + 1) * 512],
            o_ps[0:DK, :],
            rcpm,
        )

    def out_proj_j(bd, qc, shared, j):
        """Output projection for qt block j of chunk qc into a shared
        [128, 4, 768] staging tile."""
        if "osb" not in shared:
            shared["osb"] = outs.tile(
                [128, 4, D_MODEL], BF16, tag="osb", name="osb")
        osb = shared["osb"]
        qt = 4 * qc + j
        qsl = slice(qt * 128, (qt + 1) * 128)
        ps1 = px.tile([128, 512], F32, tag="ps", name="ps")
        nc.tensor.matmul(
            ps1, bd["ot"][:, 0, qsl], wo_sb[:, 0:512], start=True, stop=False)
        nc.tensor.matmul(
            ps1, bd["ot"][0:64, 1, qsl], wo_sb[0:64, 768:1280],
            start=False, stop=True)
        nc.vector.tensor_copy(osb[:, j, 0:512], ps1)
        ps2 = px.tile([128, 256], F32, tag="ps", name="ps")
        nc.tensor.matmul(
            ps2, bd["ot"][:, 0, qsl], wo_sb[:, 512:768], start=True, stop=False)
        nc.tensor.matmul(
            ps2, bd["ot"][0:64, 1, qsl], wo_sb[0:64, 1280:1536],
            start=False, stop=True)
        nc.vector.tensor_copy(osb[:, j, 512:768], ps2)

    store_eng = {"sp": nc.sync, "act": nc.scalar, "pool": nc.gpsimd}[
        store_ring]

    def out_store(qc, shared):
        if do_dma:
            store_eng.dma_start(
                out=T["out"].rearrange("(c j p) d -> c p j d", j=4, p=128)[qc],
                in_=shared["osb"],
            )

    if not do_compute:
        for _ in range(reps):
            bd = new_body()
            dummy_osb = pers.tile(
                [128, 4, D_MODEL], BF16, tag="dummy_osb", name="dummy_osb")
            nc.vector.memset(dummy_osb, 0.0)
            if do_dma:
                for qc in range(QC):
                    ({"sp": nc.sync, "act": nc.scalar,
                      "pool": nc.gpsimd}[store_ring]).dma_start(
                        out=T["out"].rearrange(
                            "(c j p) d -> c p j d", j=4, p=128)[qc],
                        in_=dummy_osb,
                    )
        return

    # ---- stream: emit score/exp groups (feeding ScalarE) with the PVs of
    # the previous group and one "filler" PE task between groups.
    # req_fillers carry a kb precedence (a projection feeding block kb
    # must be emitted before the scores that read it).
    def stream(units, req_fillers, free_fillers, group=3):
        pending_pv = []
        fi = 0
        groups = [units[j : j + group] for j in range(0, len(units), group)]
        for gi, g in enumerate(groups):
            maxkb = max(u[2] for u in g)
            while req_fillers and req_fillers[0][0] <= maxkb:
                req_fillers.pop(0)[1]()
            for pv in pending_pv:
                pv()
            for u in g:
                u[0]()
            pending_pv = [u[1] for u in g]
            if fi < len(free_fillers):
                free_fillers[fi]()
                fi += 1
        for pv in pending_pv:
            pv()
        while fi < len(free_fillers):
            free_fillers[fi]()
            fi += 1
        while req_fillers:
            req_fillers.pop(0)[1]()

    def front_tasks(bd):
        return [
            lambda: v_block(bd, 0, 0),
            lambda: proj_k_ch(bd, 0, 0),
            lambda: proj_k_ch(bd, 0, 1),
            lambda: proj_q_ch(bd, 0, 0),
            lambda: proj_q_ch(bd, 0, 1),
        ]

    bodies = {0: new_body()}
    front_emitted = set()
    prev = None  # (bd, qc, o0, o1, o2, shared) of previous chunk
    for b in range(reps):
        bd = bodies.pop(b)
        for qc in range(QC):
            if b + 1 < reps and qc == 1:
                bodies[b + 1] = new_body()
            if qc == 0 and b not in front_emitted:
                for t in front_tasks(bd):
                    t()
            qsl = slice(qc * 512, (qc + 1) * 512)
            o0 = po.tile([128, 512], F32, tag="o_ps", name="o0")
            o1 = po.tile([128, 512], F32, tag="o_ps", name="o1")
            o2 = po.tile([128, 512], F32, tag="o_ps", name="o2")
            req, free = [], []
            if qc == 0:
                for i, (off, w) in enumerate(kslices[1:], start=1):
                    b0 = off // 128
                    for kt in range(b0, (off + w) // 128):
                        req.append((kt, lambda i=i, kt=kt: v_block(bd, i, kt)))
                    req.append((b0, lambda i=i: proj_k_ch(bd, i, 0)))
                    req.append((b0, lambda i=i: proj_k_ch(bd, i, 1)))
                req.sort(key=lambda t: t[0])
            if prev is not None:
                pbd, pqc, po0, po1, po2, pshared = prev
                # normalizes must precede this chunk's first PVs (bank reuse)
                normalize(pbd, 0, pqc, po0)
                normalize(pbd, 1, pqc, po1)
                normalize(pbd, 2, pqc, po2)
                free += [
                    lambda j=j: out_proj_j(pbd, pqc, pshared, j)
                    for j in range(4)
                ]
                free.append(lambda: out_store(pqc, pshared))
            if qc + 1 < QC:
                free.append(lambda: proj_q_ch(bd, qc + 1, 0))
                free.append(lambda: proj_q_ch(bd, qc + 1, 1))
            elif b + 1 < reps:
                nb = bodies[b + 1]
                free += front_tasks(nb)
                front_emitted.add(b + 1)
            shared = {}
            units = build_units(bd, qsl, o0, o1, o2)
            stream(units, req, free)
            prev = (bd, qc, o0, o1, o2, shared)

    # tail: normalize + project + store the last chunk
    pbd, pqc, po0, po1, po2, pshared = prev
    normalize(pbd, 0, pqc, po0)
    normalize(pbd, 1, pqc, po1)
    normalize(pbd, 2, pqc, po2)
    for j in range(4):
        out_proj_j(pbd, pqc, pshared, j)
    out_store(pqc, pshared)


def build_nc(kbl=9, reps=1, mode="full", store_ring="sp"):
    nc = _Bass()
    skc = kbl * 128
    T = {
        "xtq": nc.dram_tensor("xtq", [D_MODEL, SQ], BF16, kind="ExternalInput"),
        "xtk": nc.dram_tensor("xtk", [D_MODEL, skc], BF16, kind="ExternalInput"),
        "xtv": nc.dram_tensor("xtv", [D_MODEL, skc], BF16, kind="ExternalInput"),
        "wq": nc.dram_tensor("wq", [128, FC * 2 * 128], BF16, kind="ExternalInput"),
        "wk": nc.dram_tensor("wk", [128, FC * 2 * 128], BF16, kind="ExternalInput"),
        "wv": nc.dram_tensor("wv", [128, FC * HPC * DK], BF16, kind="ExternalInput"),
        "wo": nc.dram_tensor("wo", [128, 2 * D_MODEL], BF16, kind="ExternalInput"),
        "bq": nc.dram_tensor("bq", [128, 2], F32, kind="ExternalInput"),
        "bk": nc.dram_tensor("bk", [128, 2], F32, kind="ExternalInput"),
        "bv": nc.dram_tensor("bv", [1, HPC * DK], BF16, kind="ExternalInput"),
        "m01": nc.dram_tensor("m01", [skc], F32, kind="ExternalInput"),
        "onesr": nc.dram_tensor("onesr", [1, 128], F32R, kind="ExternalInput"),
        "onesb": nc.dram_tensor("onesb", [1, 128], BF16, kind="ExternalInput"),
        "out": nc.dram_tensor("out", [SQ, D_MODEL], BF16, kind="ExternalOutput"),
    }
    with TileContext(nc) as tc, nc.allow_low_precision(reason="bf16 pipeline"):
        with (
            tc.tile_pool(name="weights", bufs=1) as wpool,
            tc.tile_pool(name="pers", bufs=2) as pers,
            tc.tile_pool(name="xq", bufs=4) as xqp,
            tc.tile_pool(name="xkv", bufs=3) as xkvp,
            tc.tile_pool(name="exps", bufs=8) as exps,
            tc.tile_pool(name="rcp", bufs=2) as rcps,
            tc.tile_pool(name="outs", bufs=2) as outs,
            tc.tile_pool(name="pp_x", bufs=2, space="PSUM") as px,
            tc.tile_pool(name="pp_st", bufs=3, space="PSUM") as pst,
            tc.tile_pool(name="pp_o", bufs=3, space="PSUM") as po,
        ):
            W = _emit_weights(nc, T, kbl, wpool)
            P = dict(pers=pers, xq=xqp, xkv=xkvp, exps=exps, rcp=rcps,
                     outs=outs, px=px, pst=pst, po=po)
            _emit_all(nc, T, kbl, W, P, reps, mode=mode,
                      store_ring=store_ring)
    return nc


# ---------------- host-side prep / gather ----------------------------------

def _bf16(a):
    import ml_dtypes
    return np.asarray(a, np.float32).astype(ml_dtypes.bfloat16)


def _prep_w(w, hd, dup):
    """lhsT layout [128 f, FC, 2, 128 m] for W rows hd (192 head dims)."""
    wh = np.asarray(w, np.float32)[hd, :]  # [192, 768]
    s1 = wh[0:128]
    if dup:
        s2 = np.concatenate([wh[128:192], wh[128:192]], axis=0)
    else:
        s2 = np.concatenate([wh[128:192], np.zeros((64, D_MODEL), np.float32)], axis=0)
    arr = np.stack([s1, s2], axis=0)  # [2, 128m, 768f]
    arr = arr.reshape(2, 128, FC, 128)  # [ch, m, fc, f]
    arr = np.ascontiguousarray(arr.transpose(3, 2, 0, 1))  # [f, fc, ch, m]
    return _bf16(arr.reshape(128, FC * 2 * 128))


def _prep_b(b, hd):
    bh = np.asarray(b, np.float32)[hd]
    c0 = bh[0:128]
    c1 = np.concatenate([bh[128:192], bh[128:192]])
    return np.ascontiguousarray(np.stack([c0, c1], axis=1))  # [128, 2] f32


def compaction(mask):
    """Per-batch live-key indices padded to a common 128-multiple."""
    mask = np.asarray(mask)
    lives = [np.where(mask[b] != 0)[0] for b in range(B)]
    n_max = max(max(len(lv) for lv in lives), 1)
    kbl = (n_max + 127) // 128
    return lives, kbl


def make_in_maps(q, k, v, mask, w_q, b_q, w_k, b_k, w_v, b_v, w_o):
    q = np.asarray(q, np.float32)
    k = np.asarray(k, np.float32)
    v = np.asarray(v, np.float32)
    lives, kbl = compaction(mask)
    skc = kbl * 128
    in_maps = []
    per_batch = []
    for b in range(B):
        lv = lives[b]
        kc = np.zeros((skc, D_MODEL), np.float32)
        vc = np.zeros((skc, D_MODEL), np.float32)
        kc[: len(lv)] = k[b][lv]
        vc[: len(lv)] = v[b][lv]
        m01 = np.zeros((skc,), np.float32)
        m01[: len(lv)] = 1.0
        per_batch.append(
            (
                _bf16(q[b].T),
                _bf16(kc.T),
                _bf16(vc.T),
                m01,
            )
        )
    w_v = np.asarray(w_v, np.float32)
    b_v = np.asarray(b_v, np.float32)
    for c in range(N_CORES):
        b = c // 4
        h0 = (c % 4) * HPC
        hd = np.arange(h0 * DK, (h0 + HPC) * DK)
        woc = np.asarray(w_o, np.float32)[:, hd]  # [768, 192]
        wot = np.ascontiguousarray(woc.T)  # [192, 768]
        wo_prep = np.zeros((128, 2 * D_MODEL), np.float32)
        wo_prep[:, 0:D_MODEL] = wot[0:128]
        wo_prep[0:64, D_MODEL:] = wot[128:192]
        # direct-V rhs layout [128 f, FC, 192 m]: wv_prep[p, fc, m] =
        # w_v[hd[m], fc*128 + p]
        wvh = w_v[hd, :]  # [192, 768]
        wv_prep = np.ascontiguousarray(
            wvh.reshape(HPC * DK, FC, 128).transpose(2, 1, 0)
        ).reshape(128, FC * HPC * DK)

        xtq, xtk, xtv, m01 = per_batch[b]
        in_maps.append(
            {
                "xtq": xtq,
                "xtk": xtk,
                "xtv": xtv,
                "wq": _prep_w(w_q, hd, True),
                "wk": _prep_w(w_k, hd, True),
                "wv": _bf16(wv_prep),
                "wo": _bf16(wo_prep),
                "bq": _prep_b(b_q, hd),
                "bk": _prep_b(b_k, hd),
                "bv": _bf16(b_v[hd][None, :]),
                "m01": m01,
                "onesr": np.ones((1, 128), np.float32),
                "onesb": _bf16(np.ones((1, 128), np.float32)),
            }
        )
    return in_maps, kbl


_NC_CACHE = {}


def kernel(q, k, v, mask, w_q, b_q, w_k, b_k, w_v, b_v, w_o, b_o, **kw):
    in_maps, kbl = make_in_maps(
        q, k, v, mask, w_q, b_q, w_k, b_k, w_v, b_v, w_o
    )
    if kbl not in _NC_CACHE:
        _NC_CACHE[kbl] = build_nc(kbl=kbl)
    nc = _NC_CACHE[kbl]
    res = run_bass_kernel_spmd(nc, in_maps, core_ids=list(range(N_CORES)))
    parts = [r["out"] for r in res.results]
    b_o = np.asarray(b_o, np.float32)
    full = np.empty((B, SQ, D_MODEL), np.float32)
    for b in range(B):
        acc = parts[4 * b].astype(np.float32)
        for c in range(4 * b + 1, 4 * b + 4):
            acc = acc + parts[c].astype(np.float32)
        full[b] = acc + b_o[None, :]
    return full
